# revision 18
# baseline (speedup 1.0000x reference)
"""AbLang2 transformer encoder layer on 8 Trainium2 NeuronCores.

Sharding: data-parallel over batch B=8 -> one batch element per core.

Per-core dataflow, built around fp8e4 DoubleRow matmuls (0.5 cycles/row,
2x128 contraction per instruction) wherever the numerics allow:

  x (bf16 copy for LN1, f32 copy for the residual) -> LN1 on DVE
  (bn_stats / 4x-mode tensor_scalar, x16 scale folded into rstd) ->
  t1 bf16 -> DMA-engine transposes (DmaTransposeAnt, [128,768] ->
  [128,6,128] blocks) -> hT8 fp8 (x16) via 2x-mode DVE copies
  -> q/k projections (fp8 DoubleRow) into f32 psums -> RoPE in
  "head-pair" layout: 4 psum-sourced tensor_tensor multiplies with
  cos/sin (dequant folded into the tables, the rotate-half slot swap
  expressed as cross-slot psum reads + a sign folded into the sin
  table) + one fp8-writing add, ops split across DVE and Pool
  -> v projection -> masked augmented V (ones column scaled s_v/16)
  -> per head: S^T fp8 DoubleRow (K=32x2), exp on ACT (scale 1/64) into
  paired E^T fp8 tiles, O^T_aug via fp8 DoubleRow over key pairs
  -> 1/s via DVE reciprocal + gpsimd partition_broadcast, O^T scaled to
  fp8 (x16) -> out-proj fp8 DoubleRow -> residual r in bf16
  -> LN2 (same DVE pipeline, t2 = 16*ln(r) bf16, 260ns 4x t-ops) ->
  DMA transposes -> h2hi fp8 = 2x copy, h2lo fp8 = compensation via
  tensor_tensor subtract -> FFN in two d_ff halves: fc1 = hi@w1hi +
  lo@w1hi (2-term DoubleRow), gelu on ACT straight to fp8,
  fc2 = g8@w2hi (1-term) -> y = r + dq*psum into f32 tiles reusing the
  x slots.

Emission order is software-pipelined by hand: engines issue strictly in
order, so RoPE tiles are interleaved between attention heads (hidden
under the ACT exp wall), FFN weight DMAs overlap attention, and the
softmax-normalization tail of head h is deferred past head h+1's work.
Offline numpy simulation of this exact quantization: rel_err = 1.77e-2.
"""

from contextlib import ExitStack

import numpy as np
import ml_dtypes

import concourse.bass as bass
import concourse.tile as tile
from concourse import bacc, mybir
from concourse.bass_utils import run_bass_kernel_spmd

F32 = mybir.dt.float32
BF16 = mybir.dt.bfloat16
F8 = mybir.dt.float8e4
NF8 = ml_dtypes.float8_e4m3
NBF = ml_dtypes.bfloat16
DR = mybir.MatmulPerfMode.DoubleRow
ALU = mybir.AluOpType
AF = mybir.ActivationFunctionType

D = 768
H = 12
HD = 64
FF = 3072
B = 8
N = 1024
P = 128
NT = N // P    # 8 token tiles
DT = D // P    # 6 d_model tiles
FT = FF // P   # 24 ffn tiles
FH = FT // 2   # 12 ffn tiles per half
QP = 96        # partitions per q/k tile (3 heads x 32 pairs)
EPS = 1e-5
SH = 16.0      # fp8 scale for LN outputs (h, h2)
SO = 16.0      # fp8 scale for O^T
SVQ = 32.0     # fp8 scale for v inside va

FC1_TERMS = 2  # h2hi@w1hi (+ h2lo@w1hi if >=2) (+ h2hi@w1lo if >=3)
FC2_TERMS = 1  # g8@w2hi (+ g8@w2lo if >=2)

last_result = None  # BassKernelResults from the most recent run (for test.py)


def _pow2_scale(absmax, target=192.0):
    if absmax <= 0:
        return 1.0
    return 2.0 ** np.floor(np.log2(target / absmax))


def _build_kernel(sc):
    """sc: dict of host-computed dequant scales baked in as immediates."""
    nc = bacc.Bacc("TRN2", target_bir_lowering=False, debug=False)

    dram = {}

    def din(name, shape, dtype=F32):
        dram[name] = nc.dram_tensor(name, list(shape), dtype, kind="ExternalInput").ap()
        return dram[name]

    din("xb", (N, D), BF16)               # bf16 copy of x for LN1
    din("x", (N, D))                      # f32 x for the residual
    if sc["use_x2"]:
        din("x2", (N, D))                 # x + bo + wo@bias_v  (residual)
    din("maskv", (P, NT))                 # key mask * v dequant scale, tiled
    din("mask1", (P, NT))                 # plain 0/1 key mask, tiled
    din("wqp", (P, DT, D), F8)            # q weights, pair layout cols per (T,i)
    din("wkp", (P, DT, D), F8)
    din("wvp", (P, DT, D), F8)            # v weights, natural out cols
    din("wop", (P, DT, D), F8)            # out-proj, oT pair-layout rows
    din("w1hi", (P, DT, FF), F8)
    if FC1_TERMS >= 3:
        din("w1lo", (P, DT, FF), F8)
    din("w2hi", (P, FT, D), F8)
    if FC2_TERMS >= 2:
        din("w2lo", (P, FT, D), F8)
    if sc["use_qkb"]:
        din("ropebq", (QP, 4, 2, N), BF16)  # rotated q bias, true scale
        din("ropebk", (QP, 4, 2, N), BF16)
    din("b1t", (P, FT))                   # fc1 bias, tiled per ff-tile
    if sc["use_b2"]:
        din("b2row", (1, D), BF16)        # b2 * s_w2 for the K=1 matmul
        din("onecol", (1, P), BF16)
    din("cosd", (P, N), BF16)             # cos * dq_qk (dequant folded in)
    din("sinds", (P, 2, N), BF16)         # [-sin, +sin] * dq_qk per slot

    y_d = nc.dram_tensor("y", [N, D], F32, kind="ExternalOutput").ap()

    with tile.TileContext(nc) as tc:
        with ExitStack() as ctx:
            _body(ctx, tc, dram, y_d, sc)
    nc.compile()
    return nc


def _body(ctx, tc, dram, y_d, sc):
    nc = tc.nc

    # ------------- pools -------------
    consts = ctx.enter_context(tc.tile_pool(name="consts", bufs=1))
    xpool = ctx.enter_context(tc.tile_pool(name="xpool", bufs=1))    # x f32 -> y
    xbpool = ctx.enter_context(tc.tile_pool(name="xbpool", bufs=1))  # x bf16
    tpool = ctx.enter_context(tc.tile_pool(name="tpool", bufs=1))    # t1/t2 bf16
    hpool = ctx.enter_context(tc.tile_pool(name="hpool", bufs=1))    # hT tiles
    qkpool = ctx.enter_context(tc.tile_pool(name="qkpool", bufs=1))  # qT/kT fp8
    rope = ctx.enter_context(tc.tile_pool(name="rope", bufs=1))      # t1c/t2s
    vpool = ctx.enter_context(tc.tile_pool(name="vpool", bufs=1))    # va pair tiles
    epool = ctx.enter_context(tc.tile_pool(name="epool", bufs=2))    # E^T pair tiles
    opool = ctx.enter_context(tc.tile_pool(name="opool", bufs=1))    # oT fp8 combined
    bcpool = ctx.enter_context(tc.tile_pool(name="bcpool", bufs=2))  # 1/s broadcast
    scpool = ctx.enter_context(tc.tile_pool(name="scpool", bufs=2))  # 1/s rows
    wf1_p = ctx.enter_context(tc.tile_pool(name="wf1_p", bufs=2))    # w1 halves (dbl)
    wf2_p = ctx.enter_context(tc.tile_pool(name="wf2_p", bufs=1))    # w2 halves
    gpool = ctx.enter_context(tc.tile_pool(name="gpool", bufs=1))    # gT half
    small = ctx.enter_context(tc.tile_pool(name="small", bufs=3))

    ps = ctx.enter_context(tc.tile_pool(name="ps", bufs=2, space="PSUM"))

    def mm_psum(name, dtype=F32):
        return ps.tile([P, N], dtype, tag="mm", name=name)

    # ------------- phase 0 emission: xb first, then hot weights -------------
    xb_tiles = []
    for t in range(NT):
        xt = xbpool.tile([P, D], BF16, tag=f"xb{t}", name=f"xb{t}")
        nc.sync.dma_start(out=xt, in_=dram["xb"][t * P:(t + 1) * P, :])
        xb_tiles.append(xt)

    eps_t = consts.tile([P, 1], F32)
    nc.vector.memset(eps_t, EPS / (SH * SH))

    def _load(nm, shape, dtype, q=None):
        t = consts.tile(list(shape), dtype, name=nm + "_sb")
        (q or nc.sync).dma_start(out=t, in_=dram[nm])
        return t

    wqp = consts.tile([P, DT, D], F8, name="wqp_sb")
    wkp = consts.tile([P, DT, D], F8, name="wkp_sb")
    wvp = consts.tile([P, DT, D], F8, name="wvp_sb")
    wop = consts.tile([P, DT, D], F8, name="wop_sb")
    for nm, t in (("wqp", wqp), ("wkp", wkp)):
        nc.gpsimd.dma_start(out=t, in_=dram[nm])

    cosd = _load("cosd", (P, N), BF16)
    sinds = _load("sinds", (P, 2, N), BF16)
    maskv = _load("maskv", (P, NT), F32, q=nc.gpsimd)
    mask1 = _load("mask1", (P, NT), F32, q=nc.gpsimd)
    b1t = _load("b1t", (P, FT), F32, q=nc.gpsimd)
    if sc["use_b2"]:
        b2row = _load("b2row", (1, D), BF16, q=nc.gpsimd)
        onecol = _load("onecol", (1, P), BF16, q=nc.gpsimd)
    if sc["use_qkb"]:
        ropebq = _load("ropebq", (QP, 4, 2, N), BF16, q=nc.gpsimd)
        ropebk = _load("ropebk", (QP, 4, 2, N), BF16, q=nc.gpsimd)

    for nm, t in (("wvp", wvp), ("wop", wop)):
        nc.gpsimd.dma_start(out=t, in_=dram[nm])

    # ------------- LN helper: t = 16 * layer_norm(src) in bf16 -------------
    def layer_norm_t(src_tiles, label):
        """DVE pipeline; the x16 fp8 pre-scale rides inside rstd."""
        ts_ = []
        for t in range(NT):
            xt = src_tiles[t]
            stats = small.tile([P, 3, 6], F32, tag="stats", name=f"st_{label}{t}")
            for g in range(3):
                nc.vector.bn_stats(out=stats[:, g, :], in_=xt[:, g * 256:(g + 1) * 256])
            mv = small.tile([P, 2], F32, tag="mv", name=f"mv_{label}{t}")
            nc.vector.bn_aggr(out=mv, in_=stats)
            # rstd16 = 16 / sqrt(var + eps) == 1 / sqrt(var/256 + eps/256)
            rstd = small.tile([P, 1], F32, tag="rstd", name=f"rs_{label}{t}")
            nc.scalar.activation(out=rstd, in_=mv[:, 1:2], func=AF.Sqrt,
                                 bias=eps_t, scale=1.0 / (SH * SH))
            nc.vector.reciprocal(out=rstd, in_=rstd)
            nmu = small.tile([P, 1], F32, tag="nmu", name=f"nmu_{label}{t}")
            nc.vector.tensor_scalar(out=nmu, in0=mv[:, 0:1], scalar1=rstd,
                                    scalar2=-1.0, op0=ALU.mult, op1=ALU.mult)
            t1 = tpool.tile([P, D], BF16, tag=f"t1_{t}", name=f"t1_{label}{t}")
            nc.vector.tensor_scalar(out=t1, in0=xt, scalar1=rstd, scalar2=nmu,
                                    op0=ALU.mult, op1=ALU.add)
            ts_.append(t1)
        return ts_

    # ---------------- LN1 -> hT8 (DMA-engine transposes) ----------------
    t1s = layer_norm_t(xb_tiles, "h")
    hTb = hpool.tile([P, DT, N], BF16, tag="hTb", name="hTb")
    for m in range(NT):
        nc.sync.dma_start_transpose(out=hTb[:, :, m * P:(m + 1) * P], in_=t1s[m])
    hT8 = hpool.tile([P, DT, N], F8, tag="hT8", name="hT8")
    for c in range(2):
        nc.vector.tensor_copy(out=hT8[:, 3 * c:3 * c + 3, :],
                              in_=hTb[:, 3 * c:3 * c + 3, :])

    # x f32 tiles for the residual: transfer rides behind the transposes
    x_tiles = []
    xsrc = "x2" if sc["use_x2"] else "x"
    for t in range(NT):
        xt = xpool.tile([P, D], F32, tag=f"x{t}", name=f"x{t}")
        nc.sync.dma_start(out=xt, in_=dram[xsrc][t * P:(t + 1) * P, :])
        x_tiles.append(xt)

    # ---------------- q/k projections + rope ----------------
    def qk_tile(wp, T, label, eng_mul=None, eng_add=None):
        """One pair-layout q/k tile [96, 2, 1024] fp8 with rope applied.

        psums stay in raw scale (SH*s_qk*q); dequant hides inside the
        cos/sin tables.  rotate_half = cross-slot read + sign in sinds.
        """
        pq = [mm_psum(f"ps_{label}{T}_{i}") for i in range(2)]
        for i in range(2):
            for t in range(DT // 2):
                for j in range(2):
                    nc.tensor.matmul(
                        pq[i][0:QP, j * 512:(j + 1) * 512],
                        wp[:, 2 * t:2 * t + 2, (T * 2 + i) * QP:(T * 2 + i + 1) * QP],
                        hT8[:, 2 * t:2 * t + 2, j * 512:(j + 1) * 512],
                        start=(t == 0), stop=(t == DT // 2 - 1), perf_mode=DR)
        em = eng_mul or (nc.vector, nc.vector, nc.vector, nc.vector)
        t1c = rope.tile([QP, 2, N], BF16, tag="t1c", name=f"t1c_{label}{T}")
        t2s = rope.tile([QP, 2, N], BF16, tag="t2s", name=f"t2s_{label}{T}")
        for i in range(2):
            em[i].tensor_tensor(out=t1c[:, i, :], in0=pq[i][0:QP, :],
                                in1=cosd[0:QP, :], op=ALU.mult)
            em[2 + i].tensor_tensor(out=t2s[:, i, :], in0=pq[1 - i][0:QP, :],
                                    in1=sinds[0:QP, i, :], op=ALU.mult)
        o = qkpool.tile([QP, 2, N], F8, tag=f"qk_{label}{T}", name=f"{label}T{T}")
        ea = eng_add or nc.vector
        if sc["use_qkb"]:
            rb = ropebq if label == "q" else ropebk
            t3 = rope.tile([QP, 2, N], BF16, tag="t3", name=f"t3_{label}{T}")
            ea.tensor_tensor(out=t3, in0=t1c, in1=t2s, op=ALU.add)
            ea.tensor_tensor(out=o, in0=t3, in1=rb[:, T, :, :], op=ALU.add)
        else:
            ea.tensor_tensor(out=o, in0=t1c, in1=t2s, op=ALU.add)
        return o

    qT, kT = [None] * 4, [None] * 4
    qT[0] = qk_tile(wqp, 0, "q")
    kT[0] = qk_tile(wkp, 0, "k")

    # ---------------- v projection -> augmented V ----------------
    va = []

    def v_tiles(us):
        for u in us:
            t = vpool.tile([P, 2, H, 80], F8, tag=f"va{u}", name=f"va{u}")
            nc.vector.memset(t[:, :, :, HD:HD + 1], sc["c0"])
            for i in range(2):
                m = 2 * u + i
                pv = mm_psum(f"ps_v{m}")
                for k in range(DT // 2):
                    for n0, nn in ((0, 512), (512, 256)):
                        nc.tensor.matmul(pv[:, n0:n0 + nn],
                                         hT8[:, 2 * k:2 * k + 2, m * P:(m + 1) * P],
                                         wvp[:, 2 * k:2 * k + 2, n0:n0 + nn],
                                         start=(k == 0), stop=(k == DT // 2 - 1),
                                         perf_mode=DR)
                nc.vector.tensor_scalar(
                    out=t[:, i, :, 0:HD],
                    in0=pv[:, 0:D].rearrange("p (h d) -> p h d", h=H),
                    scalar1=maskv[:, m:m + 1], scalar2=None, op0=ALU.mult)
                nc.vector.tensor_scalar_mul(out=t[:, i, :, HD:HD + 1],
                                            in0=t[:, i, :, HD:HD + 1],
                                            scalar1=mask1[:, m:m + 1])
            va.append(t)

    v_tiles(range(NT // 2))

    # FFN half-0 weights: transfers overlap the attention phase
    w1h = [None, None]
    w1l = [None, None]
    w2h = [None, None]
    w2l = [None, None]

    def load_ffn_half(half):
        f0 = half * FH
        w1h[half] = wf1_p.tile([P, DT, FH * P], F8, tag="w1h", name=f"w1hi_{half}")
        nc.gpsimd.dma_start(out=w1h[half], in_=dram["w1hi"][:, :, f0 * P:(f0 + FH) * P])
        if FC1_TERMS >= 3:
            w1l[half] = wf1_p.tile([P, DT, FH * P], F8, tag="w1l", name=f"w1lo_{half}")
            nc.gpsimd.dma_start(out=w1l[half], in_=dram["w1lo"][:, :, f0 * P:(f0 + FH) * P])
        w2h[half] = wf2_p.tile([P, FH, D], F8, tag="w2h", name=f"w2hi_{half}")
        nc.gpsimd.dma_start(out=w2h[half], in_=dram["w2hi"][:, f0:f0 + FH, :])
        if FC2_TERMS >= 2:
            w2l[half] = wf2_p.tile([P, FH, D], F8, tag="w2l", name=f"w2lo_{half}")
            nc.gpsimd.dma_start(out=w2l[half], in_=dram["w2lo"][:, f0:f0 + FH, :])

    load_ffn_half(0)

    # ---------------- attention ----------------
    oT8 = opool.tile([P, DT, N], F8, tag="oT8", name="oT8")
    av_ps = [None] * H
    rc_t = [None] * H

    def attend_mm(h):
        T, hh = divmod(h, 3)
        p0 = 32 * hh
        ops_t = ps.tile([P, N], F32, tag="av", name=f"av{h}")
        av_ps[h] = ops_t
        for u in range(NT // 2):
            et = epool.tile([P, 2, N], F8, tag="et", name=f"et{h}_{u}")
            for i in range(2):
                m = 2 * u + i
                pss = mm_psum(f"ps_s{h}_{m}")
                for j in range(2):
                    nc.tensor.matmul(
                        pss[:, j * 512:(j + 1) * 512],
                        kT[T][p0:p0 + 32, :, m * P:(m + 1) * P],
                        qT[T][p0:p0 + 32, :, j * 512:(j + 1) * 512],
                        start=True, stop=True, perf_mode=DR)
                nc.scalar.activation(out=et[:, i, :], in_=pss, func=AF.Exp,
                                     scale=1.0 / 64.0)
            for j in range(2):
                nc.tensor.matmul(
                    ops_t[0:HD + 1, j * 512:(j + 1) * 512],
                    va[u][:, :, h, 0:HD + 1],
                    et[:, :, j * 512:(j + 1) * 512],
                    start=(u == 0), stop=(u == NT // 2 - 1), perf_mode=DR)

    def attend_recip(h):
        rc = scpool.tile([1, N], BF16, tag="sc", name=f"sc{h}")
        with nc.allow_low_precision(reason="softmax 1/s in bf16"):
            nc.vector.reciprocal(out=rc, in_=av_ps[h][HD:HD + 1, :])
        bc = bcpool.tile([HD, N], BF16, tag="bc", name=f"bc{h}")
        nc.gpsimd.partition_broadcast(bc, rc, channels=HD)
        rc_t[h] = bc

    def attend_mul(h):
        nc.vector.tensor_mul(out=oT8[(h % 2) * HD:(h % 2) * HD + HD, h // 2, :],
                             in0=av_ps[h][0:HD, :], in1=rc_t[h])

    # software-pipelined emission: rope tiles + normalization tails hide
    # under the ACT exp wall of the attention heads
    attend_mm(0)
    qT[1] = qk_tile(wqp, 1, "q", eng_add=nc.gpsimd)
    attend_recip(0)
    attend_mm(1)
    kT[1] = qk_tile(wkp, 1, "k", eng_add=nc.gpsimd)
    attend_recip(1)
    attend_mul(0)
    attend_mm(2)
    qT[2] = qk_tile(wqp, 2, "q", eng_add=nc.gpsimd)
    attend_recip(2)
    attend_mul(1)
    attend_mm(3)
    kT[2] = qk_tile(wkp, 2, "k", eng_add=nc.gpsimd)
    attend_recip(3)
    attend_mul(2)
    attend_mm(4)
    qT[3] = qk_tile(wqp, 3, "q", eng_add=nc.gpsimd)
    attend_recip(4)
    attend_mul(3)
    attend_mm(5)
    kT[3] = qk_tile(wkp, 3, "k", eng_add=nc.gpsimd)
    attend_recip(5)
    attend_mul(4)
    for h in range(6, H):
        attend_mm(h)
        attend_recip(h)
        attend_mul(h - 1)
    attend_mul(H - 1)

    # ---------------- out-proj + residual (bf16) ----------------
    r_tiles = []
    for m in range(NT):
        po = mm_psum(f"ps_o{m}")
        for u in range(DT // 2):
            for n0, nn in ((0, 512), (512, 256)):
                nc.tensor.matmul(po[:, n0:n0 + nn],
                                 oT8[:, 2 * u:2 * u + 2, m * P:(m + 1) * P],
                                 wop[:, 2 * u:2 * u + 2, n0:n0 + nn],
                                 start=(u == 0), stop=(u == DT // 2 - 1),
                                 perf_mode=DR)
        # r reuses the xb slots (xb is dead after LN1)
        rt = xbpool.tile([P, D], BF16, tag=f"xb{m}", name=f"r{m}")
        nc.vector.scalar_tensor_tensor(out=rt, in0=po[:, 0:D], scalar=sc["dq_o"],
                                       in1=x_tiles[m], op0=ALU.mult, op1=ALU.add)
        r_tiles.append(rt)

    # ---------------- LN2 -> h2 hi/lo (DMA-engine transposes) ----------------
    t2s_ = layer_norm_t(r_tiles, "h2")
    h2Tb = hpool.tile([P, DT, N], BF16, tag="hTb", name="h2Tb")
    for m in range(NT):
        nc.sync.dma_start_transpose(out=h2Tb[:, :, m * P:(m + 1) * P], in_=t2s_[m])
    h2hi = hpool.tile([P, DT, N], F8, tag="h2hi", name="h2hi")
    h2lo = None
    if FC1_TERMS >= 2:
        h2lo = hpool.tile([P, DT, N], F8, tag="h2lo", name="h2lo")
    for c in range(DT // 2):
        nc.vector.tensor_copy(out=h2hi[:, 2 * c:2 * c + 2, :],
                              in_=h2Tb[:, 2 * c:2 * c + 2, :])
        if FC1_TERMS >= 2:
            eng = nc.gpsimd if c == 1 else nc.vector
            eng.tensor_tensor(out=h2lo[:, 2 * c:2 * c + 2, :],
                              in0=h2Tb[:, 2 * c:2 * c + 2, :],
                              in1=h2hi[:, 2 * c:2 * c + 2, :], op=ALU.subtract)

    # ---------------- FFN (two d_ff halves) ----------------
    y_tiles = []
    for half in range(2):
        f0 = half * FH
        if half == 1:
            load_ffn_half(1)
        gT = gpool.tile([P, FH, N], F8, tag="gT", name=f"gT_{half}")
        fc1_terms = [(w1h[half], h2hi), (w1h[half], h2lo), (w1l[half], h2hi)][:FC1_TERMS]
        for f in range(FH):
            pg = mm_psum(f"ps_g{half}_{f}")
            for term, (wt, rhs) in enumerate(fc1_terms):
                for t in range(DT // 2):
                    for j in range(2):
                        nc.tensor.matmul(
                            pg[:, j * 512:(j + 1) * 512],
                            wt[:, 2 * t:2 * t + 2, f * P:(f + 1) * P],
                            rhs[:, 2 * t:2 * t + 2, j * 512:(j + 1) * 512],
                            start=(term == 0 and t == 0),
                            stop=(term == len(fc1_terms) - 1 and t == DT // 2 - 1),
                            perf_mode=DR)
            nc.scalar.activation(out=gT[:, f, :], in_=pg, func=AF.Gelu,
                                 bias=b1t[:, f0 + f:f0 + f + 1], scale=sc["dq_1"])

        fc2_terms = [w2h[half], w2l[half]][:FC2_TERMS]
        for m in range(NT):
            pf = mm_psum(f"ps_f{half}_{m}")
            last_mm = not (half == 1 and sc["use_b2"])
            for term, wt in enumerate(fc2_terms):
                for u in range(FH // 2):
                    for n0, nn in ((0, 512), (512, 256)):
                        nc.tensor.matmul(
                            pf[:, n0:n0 + nn],
                            gT[:, 2 * u:2 * u + 2, m * P:(m + 1) * P],
                            wt[:, 2 * u:2 * u + 2, n0:n0 + nn],
                            start=(term == 0 and u == 0),
                            stop=(last_mm and term == len(fc2_terms) - 1
                                  and u == FH // 2 - 1),
                            perf_mode=DR)
            if half == 1 and sc["use_b2"]:
                for n0, nn in ((0, 512), (512, 256)):
                    nc.tensor.matmul(pf[:, n0:n0 + nn], onecol,
                                     b2row[:, n0:n0 + nn],
                                     start=False, stop=True)
            if half == 0:
                # y tiles reuse the x slots (x is dead after the r evac)
                yt = xpool.tile([P, D], F32, tag=f"x{m}", name=f"y{m}")
                nc.vector.scalar_tensor_tensor(out=yt, in0=pf[:, 0:D],
                                               scalar=sc["dq_2"], in1=r_tiles[m],
                                               op0=ALU.mult, op1=ALU.add)
                y_tiles.append(yt)
            else:
                nc.vector.scalar_tensor_tensor(out=y_tiles[m], in0=pf[:, 0:D],
                                               scalar=sc["dq_2"], in1=y_tiles[m],
                                               op0=ALU.mult, op1=ALU.add)
                nc.sync.dma_start(out=y_d[m * P:(m + 1) * P, :], in_=y_tiles[m])


def _host_prep(inputs):
    """Per-core input maps + dequant scale immediates."""
    g = {k: np.asarray(v) for k, v in inputs.items()}
    x = g["x"].astype(np.float32)
    pm = np.asarray(g["padding_mask"]).astype(bool)
    freqs = g["freqs"].astype(np.float32)

    ln1_w = g["ln1_w"].astype(np.float32)
    ln1_b = g["ln1_b"].astype(np.float32)
    ln2_w = g["ln2_w"].astype(np.float32)
    ln2_b = g["ln2_b"].astype(np.float32)

    # fold LN affines into the consuming weights/biases
    wq = g["wq"].astype(np.float32) * ln1_w[None, :]
    wk = g["wk"].astype(np.float32) * ln1_w[None, :]
    wv = g["wv"].astype(np.float32) * ln1_w[None, :]
    bq = g["bq"].astype(np.float32) + g["wq"].astype(np.float32) @ ln1_b
    bk = g["bk"].astype(np.float32) + g["wk"].astype(np.float32) @ ln1_b
    bias_v = g["bv"].astype(np.float32) + g["wv"].astype(np.float32) @ ln1_b
    wo = g["wo"].astype(np.float32)
    w1 = g["w1"].astype(np.float32) * ln2_w[None, :]
    b1 = g["b1"].astype(np.float32) + g["w1"].astype(np.float32) @ ln2_b
    w2 = g["w2"].astype(np.float32)
    b2 = g["b2"].astype(np.float32)

    s_qk = _pow2_scale(max(np.abs(wq).max(), np.abs(wk).max()))
    s_vw = _pow2_scale(np.abs(wv).max())
    s_ow = _pow2_scale(np.abs(wo).max())
    s_1w = _pow2_scale(np.abs(w1).max())
    s_2w = _pow2_scale(np.abs(w2).max())

    bo2 = (g["bo"].astype(np.float32) + wo @ bias_v).astype(np.float32)

    sc = dict(
        c0=float(SVQ / SO),           # va ones column; denominator scale
        dq_o=float(1.0 / (SO * s_ow)),
        dq_1=float(1.0 / (SH * s_1w)),
        dq_2=float(1.0 / s_2w),
        use_x2=bool(np.any(bo2)),
        use_b2=bool(np.any(b2)),
        use_qkb=bool(np.any(bq) or np.any(bk)),
    )
    dq_qk = 1.0 / (SH * s_qk)

    # pair-layout permutation for q/k: partition p' = hh*32 + j of tile T
    # (3 heads per 96-partition tile), slot i <-> dim d = (3T+hh)*64 + 2j + i
    qperm = np.empty((4, 2, QP), np.int64)
    for T in range(4):
        for i in range(2):
            for hh in range(3):
                for j in range(32):
                    qperm[T, i, hh * 32 + j] = (3 * T + hh) * 64 + 2 * j + i

    def kxm(w_rows_by_k, nt):  # [K_contract, M] -> [P, nt, M] (pair layout rows)
        return np.ascontiguousarray(
            w_rows_by_k.reshape(nt, P, -1).transpose(1, 0, 2))

    def qk_weight(w):
        wT = (w * s_qk).T  # [D_contract, D_out]
        cols = np.concatenate(
            [wT[:, qperm[T, i]] for T in range(4) for i in range(2)], axis=1)
        return kxm(cols, DT).astype(NF8)

    wqp = qk_weight(wq)
    wkp = qk_weight(wk)
    wvp = kxm((wv * s_vw).T, DT).astype(NF8)

    # oT pair layout rows: contraction c=(p, t) <-> d_o = (2t + p//64)*64 + p%64
    operm = np.empty((P, DT), np.int64)
    for p in range(P):
        for t in range(DT):
            operm[p, t] = (2 * t + p // 64) * 64 + (p % 64)
    woT = (wo * s_ow).T  # [d_o, m]
    wop = np.ascontiguousarray(woT[operm.reshape(-1), :].reshape(P, DT, D)).astype(NF8)

    w1s = (w1 * s_1w).T  # [D, FF]
    w1hi8 = w1s.astype(NF8)
    w1hi = kxm(w1hi8, DT)
    w2s = (w2 * s_2w).T  # [FF, D]
    w2hi8 = w2s.astype(NF8)
    w2hi = kxm(w2hi8, FT)

    # v evac: psum = (SH*h)@(wv*s_vw) -> want va = v*SVQ
    v_evac = SVQ / (SH * s_vw)

    def tile_bias(b, nt):
        return np.ascontiguousarray(b.astype(np.float32).reshape(nt, P).T)

    ang = np.outer(np.arange(N, dtype=np.float32), freqs)   # [N, 32]
    cosj = np.cos(ang).T                                     # [32, N]
    sinj = np.sin(ang).T
    cosd = (np.tile(cosj, (4, 1)) * dq_qk).astype(NBF)       # [128, N]
    sins = np.stack([-np.tile(sinj, (4, 1)) * dq_qk,
                     np.tile(sinj, (4, 1)) * dq_qk], axis=1).astype(NBF)

    shared = dict(
        wqp=wqp, wkp=wkp, wvp=wvp, wop=wop,
        w1hi=w1hi, w2hi=w2hi,
        b1t=tile_bias(b1, FT),
        cosd=cosd, sinds=sins,
    )
    if FC1_TERMS >= 3:
        shared["w1lo"] = kxm((w1s - w1hi8.astype(np.float32)).astype(NF8), DT)
    if FC2_TERMS >= 2:
        shared["w2lo"] = kxm((w2s - w2hi8.astype(np.float32)).astype(NF8), FT)
    if sc["use_b2"]:
        shared["b2row"] = np.ascontiguousarray((b2 * s_2w).astype(NBF).reshape(1, D))
        shared["onecol"] = np.ones((1, P), NBF)
    if sc["use_qkb"]:
        # rope(q + b) = rope(q) + rope(b): precompute the rotated bias table
        cos2 = np.repeat(np.cos(ang), 2, axis=-1)            # [N, 64]
        sin2 = np.repeat(np.sin(ang), 2, axis=-1)

        def rot_half(t):
            x1 = t[..., 0::2]
            x2 = t[..., 1::2]
            return np.stack((-x2, x1), axis=-1).reshape(t.shape)

        def ropeb(bvec):
            bh = bvec.reshape(H, HD)[None, :, :]             # [1, H, 64]
            rb = bh * cos2[:, None, :] + rot_half(np.broadcast_to(
                bh, (N, H, HD))) * sin2[:, None, :]          # [N, H, 64]
            out = np.zeros((QP, 4, 2, N), np.float32)
            for T in range(4):
                for i in range(2):
                    for hh in range(3):
                        for j in range(32):
                            out[hh * 32 + j, T, i, :] = rb[:, 3 * T + hh, 2 * j + i]
            return out.astype(NBF)

        shared["ropebq"] = ropeb(bq)
        shared["ropebk"] = ropeb(bk)

    in_maps = []
    for b in range(B):
        mb = np.where(pm[b], 0.0, 1.0).astype(np.float32)  # [N]
        per = dict(shared)
        per["x"] = np.ascontiguousarray(x[b])
        per["xb"] = np.ascontiguousarray(x[b].astype(NBF))
        if sc["use_x2"]:
            per["x2"] = np.ascontiguousarray(x[b] + bo2)
        per["maskv"] = np.ascontiguousarray((mb * v_evac).reshape(NT, P).T)
        per["mask1"] = np.ascontiguousarray(mb.reshape(NT, P).T)
        in_maps.append(per)
    return in_maps, sc


_nc_cache = None
_sc_cache = None


def kernel(**inputs):
    global _nc_cache, _sc_cache, last_result
    in_maps, sc = _host_prep(inputs)
    if _nc_cache is None or _sc_cache != sc:
        _nc_cache = _build_kernel(sc)
        _sc_cache = sc
    res = run_bass_kernel_spmd(_nc_cache, in_maps, list(range(B)))
    last_result = res
    y = np.stack([np.asarray(res.results[b]["y"]) for b in range(B)], axis=0)
    return y.astype(np.float32)


# revision 47
# speedup vs baseline: 1.1601x; 1.1601x over previous
"""AbLang2 transformer encoder layer on 8 Trainium2 NeuronCores.

Sharding: data-parallel over batch B=8 -> one batch element per core.

Per-core dataflow, built around fp8e4 DoubleRow matmuls (0.5 cycles/row,
2x128 contraction per instruction) wherever the numerics allow:

  x (bf16 copy for LN1, f32 copy for the residual) -> LN1 on DVE
  (bn_stats / 4x-mode tensor_scalar, x16 scale folded into rstd) ->
  t1 bf16 -> DMA-engine transposes (DmaTransposeAnt, [128,768] ->
  [128,6,128] blocks) -> hT8 fp8 (x16) via 2x-mode DVE copies
  -> q/k projections (fp8 DoubleRow) into f32 psums -> RoPE in
  "head-pair" layout: 4 psum-sourced tensor_tensor multiplies with
  cos/sin (dequant folded into the tables, the rotate-half slot swap
  expressed as cross-slot psum reads + a sign folded into the sin
  table) + one fp8-writing add, ops split across DVE and Pool
  -> v projection -> masked augmented V (ones column scaled s_v/16)
  -> per head: S^T fp8 DoubleRow (K=32x2), exp on ACT (scale 1/64) into
  paired E^T fp8 tiles, O^T_aug via fp8 DoubleRow over key pairs
  -> 1/s via DVE reciprocal + gpsimd partition_broadcast, O^T scaled to
  fp8 (x16) -> out-proj fp8 DoubleRow -> residual r in bf16
  -> LN2 (same DVE pipeline, t2 = 16*ln(r) bf16, 260ns 4x t-ops) ->
  DMA transposes -> h2hi fp8 = 2x copy, h2lo fp8 = compensation via
  tensor_tensor subtract -> FFN in two d_ff halves: fc1 = hi@w1hi +
  lo@w1hi (2-term DoubleRow), gelu on ACT straight to fp8,
  fc2 = g8@w2hi (1-term) -> y = r + dq*psum into f32 tiles reusing the
  x slots.

Emission order is software-pipelined by hand: engines issue strictly in
order, so RoPE tiles are interleaved between attention heads (hidden
under the ACT exp wall), FFN weight DMAs overlap attention, and the
softmax-normalization tail of head h is deferred past head h+1's work.
Offline numpy simulation of this exact quantization: rel_err = 1.77e-2.
"""

from contextlib import ExitStack

import numpy as np
import ml_dtypes

import concourse.bass as bass
import concourse.tile as tile
from concourse import bacc, mybir
from concourse.bass_utils import run_bass_kernel_spmd

F32 = mybir.dt.float32
BF16 = mybir.dt.bfloat16
F8 = mybir.dt.float8e4
NF8 = ml_dtypes.float8_e4m3
NBF = ml_dtypes.bfloat16
DR = mybir.MatmulPerfMode.DoubleRow
ALU = mybir.AluOpType
AF = mybir.ActivationFunctionType

D = 768
H = 12
HD = 64
FF = 3072
B = 8
N = 1024
P = 128
NT = N // P    # 8 token tiles
DT = D // P    # 6 d_model tiles
FT = FF // P   # 24 ffn tiles
FH = FT // 2   # 12 ffn tiles per half
QP = 96        # partitions per q/k tile (3 heads x 32 pairs)
EPS = 1e-5
SH = 16.0      # fp8 scale for LN outputs (h, h2)
SO = 16.0      # fp8 scale for O^T
SVQ = 32.0     # fp8 scale for v inside va

FC1_TERMS = 2  # h2hi@w1hi (+ h2lo@w1hi if >=2) (+ h2hi@w1lo if >=3)
FC2_TERMS = 1  # g8@w2hi (+ g8@w2lo if >=2)

last_result = None  # BassKernelResults from the most recent run (for test.py)


def _pow2_scale(absmax, target=192.0):
    if absmax <= 0:
        return 1.0
    return 2.0 ** np.floor(np.log2(target / absmax))


def _build_kernel(sc):
    """sc: dict of host-computed dequant scales baked in as immediates."""
    nc = bacc.Bacc("TRN2", target_bir_lowering=False, debug=False)

    dram = {}

    def din(name, shape, dtype=F32):
        dram[name] = nc.dram_tensor(name, list(shape), dtype, kind="ExternalInput").ap()
        return dram[name]

    din("xb", (N, D), BF16)               # bf16 copy of x for LN1
    din("x", (N, D))                      # f32 x for the residual
    if sc["use_x2"]:
        din("x2", (N, D))                 # x + bo + wo@bias_v  (residual)
    din("cons", (P, 2 * NT + FT))         # maskv | mask1 | b1t packed
    din("wqk", (P, 2, DT, D), F8)         # q/k weights, pair layout cols
    din("wvo", (P, 2, DT, D), F8)         # v (natural) | out-proj (pair rows)
    din("w1hi", (P, DT, FF), F8)
    if FC1_TERMS >= 3:
        din("w1lo", (P, DT, FF), F8)
    din("w2hi", (P, FT, D), F8)
    if FC2_TERMS >= 2:
        din("w2lo", (P, FT, D), F8)
    if sc["use_qkb"]:
        din("ropebq", (QP, 4, 2, N), BF16)  # rotated q bias, true scale
        din("ropebk", (QP, 4, 2, N), BF16)
    if sc["use_b2"]:
        din("b2row", (1, D), BF16)        # b2 * s_w2 for the K=1 matmul
        din("onecol", (1, P), BF16)
    din("tabs", (P, 3, N), BF16)          # cos | -sin | +sin  (* dq_qk)

    y_d = nc.dram_tensor("y", [N, D], F32, kind="ExternalOutput").ap()

    with tile.TileContext(nc) as tc:
        with ExitStack() as ctx:
            _body(ctx, tc, dram, y_d, sc)
    nc.compile()
    return nc


def _body(ctx, tc, dram, y_d, sc):
    nc = tc.nc

    # ------------- pools -------------
    consts = ctx.enter_context(tc.tile_pool(name="consts", bufs=1))
    xpool = ctx.enter_context(tc.tile_pool(name="xpool", bufs=1))    # x f32 -> y
    xbpool = ctx.enter_context(tc.tile_pool(name="xbpool", bufs=1))  # x bf16
    tpool = ctx.enter_context(tc.tile_pool(name="tpool", bufs=1))    # t1/t2 bf16
    hpool = ctx.enter_context(tc.tile_pool(name="hpool", bufs=1))    # hT tiles
    qkpool = ctx.enter_context(tc.tile_pool(name="qkpool", bufs=1))  # qT/kT fp8
    rope = ctx.enter_context(tc.tile_pool(name="rope", bufs=1))      # t1c/t2s
    vpool = ctx.enter_context(tc.tile_pool(name="vpool", bufs=1))    # va pair tiles
    epool = ctx.enter_context(tc.tile_pool(name="epool", bufs=4))    # E^T pair tiles
    opool = ctx.enter_context(tc.tile_pool(name="opool", bufs=1))    # oT fp8 combined
    bcpool = ctx.enter_context(tc.tile_pool(name="bcpool", bufs=2))  # 1/s broadcast
    scpool = ctx.enter_context(tc.tile_pool(name="scpool", bufs=2))  # 1/s rows
    wf1_p = ctx.enter_context(tc.tile_pool(name="wf1_p", bufs=2))    # w1 halves (dbl)
    wf2_p = ctx.enter_context(tc.tile_pool(name="wf2_p", bufs=1))    # w2 halves
    gpool = ctx.enter_context(tc.tile_pool(name="gpool", bufs=1))    # gT half
    small = ctx.enter_context(tc.tile_pool(name="small", bufs=3))

    ps = ctx.enter_context(tc.tile_pool(name="ps", bufs=2, space="PSUM"))

    def mm_psum(name, dtype=F32):
        return ps.tile([P, N], dtype, tag="mm", name=name)

    # ------------- phase 0 emission: xb first, then hot weights -------------
    # consolidated DMAs: each HWDGE queue only sustains ~2 in-flight
    # transfers, so many small DMAs serialize at several us apiece.
    xbh = []
    for h in range(2):
        xt = xbpool.tile([P, 4, D], BF16, tag=f"xbh{h}", name=f"xbh{h}")
        nc.sync.dma_start(
            out=xt, in_=dram["xb"][4 * h * P:4 * (h + 1) * P, :].rearrange(
                "(m p) d -> p m d", p=P))
        xbh.append(xt)
    xb_tiles = [xbh[t // 4][:, t % 4, :] for t in range(NT)]

    eps_t = consts.tile([P, 1], F32)
    nc.vector.memset(eps_t, EPS / (SH * SH))

    def _load(nm, shape, dtype, q=None):
        t = consts.tile(list(shape), dtype, name=nm + "_sb")
        (q or nc.sync).dma_start(out=t, in_=dram[nm])
        return t

    wqk = consts.tile([P, 2, DT, D], F8, name="wqk_sb")
    with tc.tile_wait_until(0.006):
        nc.gpsimd.dma_start(out=wqk, in_=dram["wqk"])
    wqp, wkp = wqk[:, 0, :, :], wqk[:, 1, :, :]

    tabs = _load("tabs", (P, 3, N), BF16)
    cosd = tabs[:, 0, :]
    sinds = tabs[:, 1:3, :]
    cons = _load("cons", (P, 2 * NT + FT), F32)
    maskv = cons[:, 0:NT]
    mask1 = cons[:, NT:2 * NT]
    b1t = cons[:, 2 * NT:]
    if sc["use_b2"]:
        b2row = _load("b2row", (1, D), BF16)
        onecol = _load("onecol", (1, P), BF16)
    if sc["use_qkb"]:
        ropebq = _load("ropebq", (QP, 4, 2, N), BF16, q=nc.gpsimd)
        ropebk = _load("ropebk", (QP, 4, 2, N), BF16, q=nc.gpsimd)

    wvo = consts.tile([P, 2, DT, D], F8, name="wvo_sb")
    with tc.tile_wait_until(0.03):
        nc.gpsimd.dma_start(out=wvo, in_=dram["wvo"])
    wvp, wop = wvo[:, 0, :, :], wvo[:, 1, :, :]

    # ------------- LN helper: t = 16 * layer_norm(src) in bf16 -------------
    def layer_norm_t(src_tiles, label):
        """DVE pipeline; the x16 fp8 pre-scale rides inside rstd."""
        ts_ = []
        for t in range(NT):
            xt = src_tiles[t]
            stats = small.tile([P, 3, 6], F32, tag="stats", name=f"st_{label}{t}")
            for g in range(3):
                nc.vector.bn_stats(out=stats[:, g, :], in_=xt[:, g * 256:(g + 1) * 256])
            mv = small.tile([P, 2], F32, tag="mv", name=f"mv_{label}{t}")
            nc.vector.bn_aggr(out=mv, in_=stats)
            # rstd16 = 16 / sqrt(var + eps) == 1 / sqrt(var/256 + eps/256)
            rstd = small.tile([P, 1], F32, tag="rstd", name=f"rs_{label}{t}")
            nc.scalar.activation(out=rstd, in_=mv[:, 1:2], func=AF.Sqrt,
                                 bias=eps_t, scale=1.0 / (SH * SH))
            nc.vector.reciprocal(out=rstd, in_=rstd)
            nmu = small.tile([P, 1], F32, tag="nmu", name=f"nmu_{label}{t}")
            nc.vector.tensor_scalar(out=nmu, in0=mv[:, 0:1], scalar1=rstd,
                                    scalar2=-1.0, op0=ALU.mult, op1=ALU.mult)
            t1 = tpool.tile([P, D], BF16, tag=f"t1_{t}", name=f"t1_{label}{t}")
            nc.vector.tensor_scalar(out=t1, in0=xt, scalar1=rstd, scalar2=nmu,
                                    op0=ALU.mult, op1=ALU.add)
            ts_.append(t1)
        return ts_

    # ---------------- LN1 -> hT8 (DMA-engine transposes) ----------------
    t1s = layer_norm_t(xb_tiles, "h")
    hTb = hpool.tile([P, DT, N], BF16, tag="hTb", name="hTb")
    for m in range(NT):
        # ACT's HWDGE queue: keeps the transposes off the busy sync queue
        nc.scalar.dma_start_transpose(out=hTb[:, :, m * P:(m + 1) * P], in_=t1s[m])

    # x f32 for the residual: one DMA, held back to ~22us by a manual
    # wait so the scheduler can't hoist its 9us transfer into the lead-in
    xsrc = "x2" if sc["use_x2"] else "x"
    x_all = xpool.tile([P, NT, D], F32, tag="xall", name="x_all")
    with tc.tile_wait_until(0.022):
        nc.scalar.dma_start(out=x_all,
                            in_=dram[xsrc].rearrange("(m p) d -> p m d", p=P))
    x_tiles = [x_all[:, m, :] for m in range(NT)]

    hT8 = hpool.tile([P, DT, N], F8, tag="hT8", name="hT8")
    for c in range(DT // 2):
        # d-pair chunks so the q/k projections chase the quantize
        nc.vector.tensor_copy(out=hT8[:, 2 * c:2 * c + 2, :],
                              in_=hTb[:, 2 * c:2 * c + 2, :])

    # ---------------- q/k projections + rope ----------------
    def qk_tile(wp, T, label, eng_add=None, chunks=None):
        """One pair-layout q/k tile [96, 2, 1024] fp8 with rope applied.

        psums stay in raw scale (SH*s_qk*q); dequant hides inside the
        cos/sin tables.  rotate_half = cross-slot read + sign in sinds.
        The psums borrow the "av" ring so slow rope-mul drains stall the
        (slack-rich) AV path instead of the exp-feeding S psums.
        """
        pq = [ps.tile([P, N], F32, tag="av", name=f"ps_{label}{T}_{i}")
              for i in range(2)]
        for i in range(2):
            for t in range(DT // 2):
                for j in range(2):
                    nc.tensor.matmul(
                        pq[i][0:QP, j * 512:(j + 1) * 512],
                        wp[:, 2 * t:2 * t + 2, (T * 2 + i) * QP:(T * 2 + i + 1) * QP],
                        hT8[:, 2 * t:2 * t + 2, j * 512:(j + 1) * 512],
                        start=(t == 0), stop=(t == DT // 2 - 1), perf_mode=DR)
        t1c = rope.tile([QP, 2, N], BF16, tag="t1c", name=f"t1c_{label}{T}")
        t2s = rope.tile([QP, 2, N], BF16, tag="t2s", name=f"t2s_{label}{T}")
        o = qkpool.tile([QP, 2, N], F8, tag=f"qk_{label}{T}", name=f"{label}T{T}")
        ea = eng_add or nc.gpsimd
        rb = None
        if sc["use_qkb"]:
            rb = ropebq if label == "q" else ropebk
        for c0, cn in chunks or [(0, N)]:
            for i in range(2):
                # psum-sourced ops must be on DVE (gpsimd cannot read PSUM);
                # high priority so the borrowed av-ring psums drain fast
                with tc.high_priority():
                    nc.vector.tensor_tensor(out=t1c[:, i, c0:c0 + cn],
                                            in0=pq[i][0:QP, c0:c0 + cn],
                                            in1=cosd[0:QP, c0:c0 + cn], op=ALU.mult)
                    nc.vector.tensor_tensor(out=t2s[:, i, c0:c0 + cn],
                                            in0=pq[1 - i][0:QP, c0:c0 + cn],
                                            in1=sinds[0:QP, i, c0:c0 + cn], op=ALU.mult)
            if rb is not None:
                t3 = rope.tile([QP, 2, N], BF16, tag="t3", name=f"t3_{label}{T}")
                ea.tensor_tensor(out=t3[:, :, c0:c0 + cn],
                                 in0=t1c[:, :, c0:c0 + cn],
                                 in1=t2s[:, :, c0:c0 + cn], op=ALU.add)
                ea.tensor_tensor(out=o[:, :, c0:c0 + cn],
                                 in0=t3[:, :, c0:c0 + cn],
                                 in1=rb[:, T, :, c0:c0 + cn], op=ALU.add)
            else:
                ea.tensor_tensor(out=o[:, :, c0:c0 + cn],
                                 in0=t1c[:, :, c0:c0 + cn],
                                 in1=t2s[:, :, c0:c0 + cn], op=ALU.add)
        return o

    qT, kT = [None] * 4, [None] * 4
    qT[0] = qk_tile(wqp, 0, "q", eng_add=nc.vector)
    kT[0] = qk_tile(wkp, 0, "k", eng_add=nc.vector,
                    chunks=[(0, 256), (256, 256), (512, 256), (768, 256)])

    # ---------------- v projection -> augmented V ----------------
    va = []

    def v_tiles(us):
        for u in us:
            t = vpool.tile([P, 2, H, 80], F8, tag=f"va{u}", name=f"va{u}")
            nc.vector.memset(t[:, :, :, HD:HD + 1], sc["c0"])
            for i in range(2):
                m = 2 * u + i
                pv = mm_psum(f"ps_v{m}")
                for k in range(DT // 2):
                    for n0, nn in ((0, 512), (512, 256)):
                        nc.tensor.matmul(pv[:, n0:n0 + nn],
                                         hT8[:, 2 * k:2 * k + 2, m * P:(m + 1) * P],
                                         wvp[:, 2 * k:2 * k + 2, n0:n0 + nn],
                                         start=(k == 0), stop=(k == DT // 2 - 1),
                                         perf_mode=DR)
                nc.vector.tensor_scalar(
                    out=t[:, i, :, 0:HD],
                    in0=pv[:, 0:D].rearrange("p (h d) -> p h d", h=H),
                    scalar1=maskv[:, m:m + 1], scalar2=None, op0=ALU.mult)
                nc.vector.tensor_scalar_mul(out=t[:, i, :, HD:HD + 1],
                                            in0=t[:, i, :, HD:HD + 1],
                                            scalar1=mask1[:, m:m + 1])
            va.append(t)

    v_tiles(range(NT // 2))

    # FFN half-0 weights: transfers overlap the attention phase
    w1h = [None, None]
    w1l = [None, None]
    w2h = [None, None]
    w2l = [None, None]

    def load_ffn_half(half):
        f0 = half * FH
        w1h[half] = wf1_p.tile([P, DT, FH * P], F8, tag="w1h", name=f"w1hi_{half}")
        nc.gpsimd.dma_start(out=w1h[half], in_=dram["w1hi"][:, :, f0 * P:(f0 + FH) * P])
        if FC1_TERMS >= 3:
            w1l[half] = wf1_p.tile([P, DT, FH * P], F8, tag="w1l", name=f"w1lo_{half}")
            nc.gpsimd.dma_start(out=w1l[half], in_=dram["w1lo"][:, :, f0 * P:(f0 + FH) * P])
        w2h[half] = wf2_p.tile([P, FH, D], F8, tag="w2h", name=f"w2hi_{half}")
        nc.gpsimd.dma_start(out=w2h[half], in_=dram["w2hi"][:, f0:f0 + FH, :])
        if FC2_TERMS >= 2:
            w2l[half] = wf2_p.tile([P, FH, D], F8, tag="w2l", name=f"w2lo_{half}")
            nc.gpsimd.dma_start(out=w2l[half], in_=dram["w2lo"][:, f0:f0 + FH, :])

    # ---------------- attention ----------------
    oT8 = opool.tile([P, DT, N], F8, tag="oT8", name="oT8")
    av_ps = [None] * H
    rc_t = [None] * H

    def attend_mm(h):
        T, hh = divmod(h, 3)
        p0 = 32 * hh
        ops_t = ps.tile([P, N], F32, tag="av", name=f"av{h}")
        av_ps[h] = ops_t
        for u in range(NT // 2):
            et = epool.tile([P, 2, N], F8, tag="et", name=f"et{h}_{u}")
            for i in range(2):
                m = 2 * u + i
                pss = mm_psum(f"ps_s{h}_{m}")
                for j in range(2):
                    nc.tensor.matmul(
                        pss[:, j * 512:(j + 1) * 512],
                        kT[T][p0:p0 + 32, :, m * P:(m + 1) * P],
                        qT[T][p0:p0 + 32, :, j * 512:(j + 1) * 512],
                        start=True, stop=True, perf_mode=DR)
                nc.scalar.activation(out=et[:, i, :], in_=pss, func=AF.Exp,
                                     scale=1.0 / 64.0)
            for j in range(2):
                nc.tensor.matmul(
                    ops_t[0:HD + 1, j * 512:(j + 1) * 512],
                    va[u][:, :, h, 0:HD + 1],
                    et[:, :, j * 512:(j + 1) * 512],
                    start=(u == 0), stop=(u == NT // 2 - 1), perf_mode=DR)

    def attend_recip(h):
        rc = scpool.tile([1, N], BF16, tag="sc", name=f"sc{h}")
        with nc.allow_low_precision(reason="softmax 1/s in bf16"):
            nc.vector.reciprocal(out=rc, in_=av_ps[h][HD:HD + 1, :])
        bc = bcpool.tile([HD, N], BF16, tag="bc", name=f"bc{h}")
        nc.gpsimd.partition_broadcast(bc, rc, channels=HD)
        rc_t[h] = bc

    def attend_mul(h):
        nc.vector.tensor_mul(out=oT8[(h % 2) * HD:(h % 2) * HD + HD, h // 2, :],
                             in0=av_ps[h][0:HD, :], in1=rc_t[h])

    # software-pipelined emission: rope tiles + normalization tails hide
    # under the ACT exp wall of the attention heads
    attend_mm(0)
    qT[1] = qk_tile(wqp, 1, "q", eng_add=nc.gpsimd)
    attend_recip(0)
    attend_mm(1)
    kT[1] = qk_tile(wkp, 1, "k", eng_add=nc.gpsimd)
    attend_recip(1)
    attend_mul(0)
    attend_mm(2)
    with tc.tile_wait_until(0.09):
        load_ffn_half(0)   # transfers start after the lead-in DMAs clear
    qT[2] = qk_tile(wqp, 2, "q", eng_add=nc.gpsimd)
    attend_recip(2)
    attend_mul(1)
    attend_mm(3)
    kT[2] = qk_tile(wkp, 2, "k", eng_add=nc.gpsimd)
    attend_recip(3)
    attend_mul(2)
    attend_mm(4)
    qT[3] = qk_tile(wqp, 3, "q", eng_add=nc.gpsimd)
    attend_recip(4)
    attend_mul(3)
    attend_mm(5)
    kT[3] = qk_tile(wkp, 3, "k", eng_add=nc.gpsimd)
    attend_recip(5)
    attend_mul(4)
    for h in range(6, H):
        attend_mm(h)
        attend_recip(h)
        attend_mul(h - 1)
    attend_mul(H - 1)

    # ---------------- out-proj + residual (bf16) ----------------
    r_tiles = []
    for m in range(NT):
        po = mm_psum(f"ps_o{m}")
        for u in range(DT // 2):
            for n0, nn in ((0, 512), (512, 256)):
                nc.tensor.matmul(po[:, n0:n0 + nn],
                                 oT8[:, 2 * u:2 * u + 2, m * P:(m + 1) * P],
                                 wop[:, 2 * u:2 * u + 2, n0:n0 + nn],
                                 start=(u == 0), stop=(u == DT // 2 - 1),
                                 perf_mode=DR)
        # r reuses the xb slots (xb is dead after LN1)
        rt = xbpool.tile([P, D], BF16, tag=f"xb{m}", name=f"r{m}")
        nc.vector.scalar_tensor_tensor(out=rt, in0=po[:, 0:D], scalar=sc["dq_o"],
                                       in1=x_tiles[m], op0=ALU.mult, op1=ALU.add)
        r_tiles.append(rt)

    # ---------------- LN2 -> h2 hi/lo (DMA-engine transposes) ----------------
    t2s_ = layer_norm_t(r_tiles, "h2")
    h2Tb = hpool.tile([P, DT, N], BF16, tag="hTb", name="h2Tb")
    for m in range(NT):
        nc.sync.dma_start_transpose(out=h2Tb[:, :, m * P:(m + 1) * P], in_=t2s_[m])
    h2hi = hpool.tile([P, DT, N], F8, tag="h2hi", name="h2hi")
    h2lo = None
    if FC1_TERMS >= 2:
        h2lo = hpool.tile([P, DT, N], F8, tag="h2lo", name="h2lo")
    for c in range(DT // 2):
        nc.vector.tensor_copy(out=h2hi[:, 2 * c:2 * c + 2, :],
                              in_=h2Tb[:, 2 * c:2 * c + 2, :])
        if FC1_TERMS >= 2:
            eng = nc.gpsimd if c == 1 else nc.vector
            eng.tensor_tensor(out=h2lo[:, 2 * c:2 * c + 2, :],
                              in0=h2Tb[:, 2 * c:2 * c + 2, :],
                              in1=h2hi[:, 2 * c:2 * c + 2, :],
                              op=ALU.subtract)

    # ---------------- FFN (two d_ff halves) ----------------
    # y reuses x_all's memory (tag ring; x is dead after the r evacs)
    y_all = xpool.tile([P, NT, D], F32, tag="xall", name="y_all")
    for half in range(2):
        f0 = half * FH
        if half == 1:
            with tc.tile_wait_until(0.10):
                load_ffn_half(1)
        gT = gpool.tile([P, FH, N], F8, tag="gT", name=f"gT_{half}")
        fc1_terms = [(w1h[half], h2hi), (w1h[half], h2lo), (w1l[half], h2hi)][:FC1_TERMS]
        for f in range(FH):
            pg = mm_psum(f"ps_g{half}_{f}")
            for term, (wt, rhs) in enumerate(fc1_terms):
                for t in range(DT // 2):
                    for j in range(2):
                        nc.tensor.matmul(
                            pg[:, j * 512:(j + 1) * 512],
                            wt[:, 2 * t:2 * t + 2, f * P:(f + 1) * P],
                            rhs[:, 2 * t:2 * t + 2, j * 512:(j + 1) * 512],
                            start=(term == 0 and t == 0),
                            stop=(term == len(fc1_terms) - 1 and t == DT // 2 - 1),
                            perf_mode=DR)
            nc.scalar.activation(out=gT[:, f, :], in_=pg, func=AF.Gelu,
                                 bias=b1t[:, f0 + f:f0 + f + 1], scale=sc["dq_1"])

        fc2_terms = [w2h[half], w2l[half]][:FC2_TERMS]
        for m in range(NT):
            pf = mm_psum(f"ps_f{half}_{m}")
            last_mm = not (half == 1 and sc["use_b2"])
            for term, wt in enumerate(fc2_terms):
                for u in range(FH // 2):
                    for n0, nn in ((0, 512), (512, 256)):
                        nc.tensor.matmul(
                            pf[:, n0:n0 + nn],
                            gT[:, 2 * u:2 * u + 2, m * P:(m + 1) * P],
                            wt[:, 2 * u:2 * u + 2, n0:n0 + nn],
                            start=(term == 0 and u == 0),
                            stop=(last_mm and term == len(fc2_terms) - 1
                                  and u == FH // 2 - 1),
                            perf_mode=DR)
            if half == 1 and sc["use_b2"]:
                for n0, nn in ((0, 512), (512, 256)):
                    nc.tensor.matmul(pf[:, n0:n0 + nn], onecol,
                                     b2row[:, n0:n0 + nn],
                                     start=False, stop=True)
            if half == 0:
                nc.vector.scalar_tensor_tensor(out=y_all[:, m, :], in0=pf[:, 0:D],
                                               scalar=sc["dq_2"], in1=r_tiles[m],
                                               op0=ALU.mult, op1=ALU.add)
            else:
                nc.vector.scalar_tensor_tensor(out=y_all[:, m, :], in0=pf[:, 0:D],
                                               scalar=sc["dq_2"], in1=y_all[:, m, :],
                                               op0=ALU.mult, op1=ALU.add)
                nc.sync.dma_start(out=y_d[m * P:(m + 1) * P, :], in_=y_all[:, m, :])


def _host_prep(inputs):
    """Per-core input maps + dequant scale immediates."""
    g = {k: np.asarray(v) for k, v in inputs.items()}
    x = g["x"].astype(np.float32)
    pm = np.asarray(g["padding_mask"]).astype(bool)
    freqs = g["freqs"].astype(np.float32)

    ln1_w = g["ln1_w"].astype(np.float32)
    ln1_b = g["ln1_b"].astype(np.float32)
    ln2_w = g["ln2_w"].astype(np.float32)
    ln2_b = g["ln2_b"].astype(np.float32)

    # fold LN affines into the consuming weights/biases
    wq = g["wq"].astype(np.float32) * ln1_w[None, :]
    wk = g["wk"].astype(np.float32) * ln1_w[None, :]
    wv = g["wv"].astype(np.float32) * ln1_w[None, :]
    bq = g["bq"].astype(np.float32) + g["wq"].astype(np.float32) @ ln1_b
    bk = g["bk"].astype(np.float32) + g["wk"].astype(np.float32) @ ln1_b
    bias_v = g["bv"].astype(np.float32) + g["wv"].astype(np.float32) @ ln1_b
    wo = g["wo"].astype(np.float32)
    w1 = g["w1"].astype(np.float32) * ln2_w[None, :]
    b1 = g["b1"].astype(np.float32) + g["w1"].astype(np.float32) @ ln2_b
    w2 = g["w2"].astype(np.float32)
    b2 = g["b2"].astype(np.float32)

    s_qk = _pow2_scale(max(np.abs(wq).max(), np.abs(wk).max()))
    s_vw = _pow2_scale(np.abs(wv).max())
    s_ow = _pow2_scale(np.abs(wo).max())
    s_1w = _pow2_scale(np.abs(w1).max())
    s_2w = _pow2_scale(np.abs(w2).max())

    bo2 = (g["bo"].astype(np.float32) + wo @ bias_v).astype(np.float32)

    sc = dict(
        c0=float(SVQ / SO),           # va ones column; denominator scale
        dq_o=float(1.0 / (SO * s_ow)),
        dq_1=float(1.0 / (SH * s_1w)),
        dq_2=float(1.0 / s_2w),
        use_x2=bool(np.any(bo2)),
        use_b2=bool(np.any(b2)),
        use_qkb=bool(np.any(bq) or np.any(bk)),
    )
    dq_qk = 1.0 / (SH * s_qk)

    # pair-layout permutation for q/k: partition p' = hh*32 + j of tile T
    # (3 heads per 96-partition tile), slot i <-> dim d = (3T+hh)*64 + 2j + i
    qperm = np.empty((4, 2, QP), np.int64)
    for T in range(4):
        for i in range(2):
            for hh in range(3):
                for j in range(32):
                    qperm[T, i, hh * 32 + j] = (3 * T + hh) * 64 + 2 * j + i

    def kxm(w_rows_by_k, nt):  # [K_contract, M] -> [P, nt, M] (pair layout rows)
        return np.ascontiguousarray(
            w_rows_by_k.reshape(nt, P, -1).transpose(1, 0, 2))

    def qk_weight(w):
        wT = (w * s_qk).T  # [D_contract, D_out]
        cols = np.concatenate(
            [wT[:, qperm[T, i]] for T in range(4) for i in range(2)], axis=1)
        return kxm(cols, DT).astype(NF8)

    wqp = qk_weight(wq)
    wkp = qk_weight(wk)
    wvp = kxm((wv * s_vw).T, DT).astype(NF8)

    # oT pair layout rows: contraction c=(p, t) <-> d_o = (2t + p//64)*64 + p%64
    operm = np.empty((P, DT), np.int64)
    for p in range(P):
        for t in range(DT):
            operm[p, t] = (2 * t + p // 64) * 64 + (p % 64)
    woT = (wo * s_ow).T  # [d_o, m]
    wop = np.ascontiguousarray(woT[operm.reshape(-1), :].reshape(P, DT, D)).astype(NF8)

    w1s = (w1 * s_1w).T  # [D, FF]
    w1hi8 = w1s.astype(NF8)
    w1hi = kxm(w1hi8, DT)
    w2s = (w2 * s_2w).T  # [FF, D]
    w2hi8 = w2s.astype(NF8)
    w2hi = kxm(w2hi8, FT)

    # v evac: psum = (SH*h)@(wv*s_vw) -> want va = v*SVQ
    v_evac = SVQ / (SH * s_vw)

    def tile_bias(b, nt):
        return np.ascontiguousarray(b.astype(np.float32).reshape(nt, P).T)

    ang = np.outer(np.arange(N, dtype=np.float32), freqs)   # [N, 32]
    cosj = np.cos(ang).T                                     # [32, N]
    sinj = np.sin(ang).T
    cosd = (np.tile(cosj, (4, 1)) * dq_qk).astype(NBF)       # [128, N]
    sins = np.stack([-np.tile(sinj, (4, 1)) * dq_qk,
                     np.tile(sinj, (4, 1)) * dq_qk], axis=1).astype(NBF)

    shared = dict(
        wqk=np.ascontiguousarray(np.stack([wqp, wkp], axis=1)),
        wvo=np.ascontiguousarray(np.stack([wvp, wop], axis=1)),
        w1hi=w1hi, w2hi=w2hi,
        tabs=np.ascontiguousarray(np.stack(
            [cosd, sins[:, 0, :], sins[:, 1, :]], axis=1)),
    )
    if FC1_TERMS >= 3:
        shared["w1lo"] = kxm((w1s - w1hi8.astype(np.float32)).astype(NF8), DT)
    if FC2_TERMS >= 2:
        shared["w2lo"] = kxm((w2s - w2hi8.astype(np.float32)).astype(NF8), FT)
    if sc["use_b2"]:
        shared["b2row"] = np.ascontiguousarray((b2 * s_2w).astype(NBF).reshape(1, D))
        shared["onecol"] = np.ones((1, P), NBF)
    if sc["use_qkb"]:
        # rope(q + b) = rope(q) + rope(b): precompute the rotated bias table
        cos2 = np.repeat(np.cos(ang), 2, axis=-1)            # [N, 64]
        sin2 = np.repeat(np.sin(ang), 2, axis=-1)

        def rot_half(t):
            x1 = t[..., 0::2]
            x2 = t[..., 1::2]
            return np.stack((-x2, x1), axis=-1).reshape(t.shape)

        def ropeb(bvec):
            bh = bvec.reshape(H, HD)[None, :, :]             # [1, H, 64]
            rb = bh * cos2[:, None, :] + rot_half(np.broadcast_to(
                bh, (N, H, HD))) * sin2[:, None, :]          # [N, H, 64]
            out = np.zeros((QP, 4, 2, N), np.float32)
            for T in range(4):
                for i in range(2):
                    for hh in range(3):
                        for j in range(32):
                            out[hh * 32 + j, T, i, :] = rb[:, 3 * T + hh, 2 * j + i]
            return out.astype(NBF)

        shared["ropebq"] = ropeb(bq)
        shared["ropebk"] = ropeb(bk)

    in_maps = []
    for b in range(B):
        mb = np.where(pm[b], 0.0, 1.0).astype(np.float32)  # [N]
        per = dict(shared)
        per["x"] = np.ascontiguousarray(x[b])
        per["xb"] = np.ascontiguousarray(x[b].astype(NBF))
        if sc["use_x2"]:
            per["x2"] = np.ascontiguousarray(x[b] + bo2)
        per["cons"] = np.ascontiguousarray(np.concatenate(
            [(mb * v_evac).reshape(NT, P).T, mb.reshape(NT, P).T,
             tile_bias(b1, FT)], axis=1))
        in_maps.append(per)
    return in_maps, sc


_nc_cache = None
_sc_cache = None


def kernel(**inputs):
    global _nc_cache, _sc_cache, last_result
    in_maps, sc = _host_prep(inputs)
    if _nc_cache is None or _sc_cache != sc:
        _nc_cache = _build_kernel(sc)
        _sc_cache = sc
    res = run_bass_kernel_spmd(_nc_cache, in_maps, list(range(B)))
    last_result = res
    y = np.stack([np.asarray(res.results[b]["y"]) for b in range(B)], axis=0)
    return y.astype(np.float32)
